# revision 37
# baseline (speedup 1.0000x reference)
"""Multi-head attention (B=4, N=2048, D=1024, H=16, DH=64) on 8 TRN2 NeuronCores.

Sharding: (batch x head-group) grid = 4x2 = 8 cores. Core c handles batch
b=c//2 and heads hg*8..hg*8+8 (hg=c%2). Each core projects q/k/v for its
OWN 8 heads over the FULL sequence (no k/v collective at all), runs
attention for those heads over all 2048 x 2048 (q x k), and finally the
cores of a batch exchange attention outputs (1MB bf16 AllGather pair)
so each can run the output projection for a disjoint seq half with the
full 1024-dim contraction.

Host-side seq-half swap makes the SPMD program core-uniform: each core's
LOCAL seq order is [half-it-sends, half-it-keeps]; attention is permutation
invariant over keys, so only inputs/outputs are permuted.

Sim matmuls use K=64 row tiling (tile_position (0,0)/(64,0) auto-derived):
the two heads of a column group run CONCURRENTLY in the PE array - no
zero-padding waste. The av matmul uses the 65-column augmented-v trick
(64 v dims + ones column) to get softmax denominators for free.

The scalar engine (exp over 33.5M logits/core at ~1 elem/lane/cycle) is
the pacer; emission order keeps >=1 un-exp'd sim block queued at all
times while proj/av/out-proj fill the PE.

Per-core layouts (bf16 compute, f32 accumulation):
  xt   [128, 8, 2048]  x[b].T full seq (proj moving / v stationary)
  qT   [128, 4, 2048]  local q dims (2 heads per col-group) x seq
  kT   [128, 4, 2048]  same for k
  vaug [128, 16, 520]  seq tiles x (8 heads x 65), 65th col = 1
  aoT  [128, 4, 2048]  attention out dims x seq; cols 0:1024 = send half
"""
import sys

sys.path.insert(0, "/opt/trn_rl_repo")

import numpy as np
import ml_dtypes

import concourse.bass as bass
import concourse.bacc as bacc
import concourse.mybir as mybir
import concourse.tile as tile
from contextlib import ExitStack

BF = mybir.dt.bfloat16
F32 = mybir.dt.float32
bf16 = ml_dtypes.bfloat16

P = 128
B, N, D = 4, 2048, 1024
H, DH = 16, 64
HL = 8            # local heads per core
G = 4             # head-pair groups (col-tiles of the local 512 q/k dims)
DT = 8            # contraction tiles over D
STK = 16          # seq tiles of 128
F = 512           # matmul moving free dim
NQ = N // 2       # seq positions output per core (keep half)
VW = HL * (DH + 1)     # 520 vaug payload width
EXPF = mybir.ActivationFunctionType.Exp
SCALE = DH ** -0.5
GROUPS = [[0, 1], [2, 3], [4, 5], [6, 7]]

_CACHED_NC = None


def build_nc(exchange=True):
    """exchange=True: pairwise AllGather of aoT halves (HW).
    exchange=False: loopback (recv := own send half) for CoreSim."""
    nc = bacc.Bacc("TRN2", debug=False, num_devices=8)
    xt_d = nc.dram_tensor("xt", [D, N], BF, kind="ExternalInput")
    cosT_d = nc.dram_tensor("cosT", [P, N], BF, kind="ExternalInput")
    sinT_d = nc.dram_tensor("sinT", [P, N], BF, kind="ExternalInput")
    cosv_d = nc.dram_tensor("cosv", [N, DH], BF, kind="ExternalInput")
    sinv_d = nc.dram_tensor("sinv", [N, DH], BF, kind="ExternalInput")
    smat_d = nc.dram_tensor("smat", [P, P], BF, kind="ExternalInput")
    nz_d = nc.dram_tensor("nz", [1, 2], BF, kind="ExternalInput")
    wq_d = nc.dram_tensor("wq", [D, HL * DH], BF, kind="ExternalInput")
    wk_d = nc.dram_tensor("wk", [D, HL * DH], BF, kind="ExternalInput")
    wv_d = nc.dram_tensor("wv", [D, HL * DH], BF, kind="ExternalInput")
    wout_d = nc.dram_tensor("wout", [D, D], BF, kind="ExternalInput")
    out_d = nc.dram_tensor("out", [NQ, D], F32, kind="ExternalOutput")

    with tile.TileContext(nc) as tc, ExitStack() as pc:
        pers = pc.enter_context(tc.tile_pool(name="pers", bufs=1))
        xt = pers.tile([P, DT, N], BF, name="xt")
        qT = pers.tile([P, G, N], BF, name="qT")
        kT = pers.tile([P, G, N], BF, name="kT")
        vaug = pers.tile([P, STK, VW], BF, name="vaug")
        aoT = pers.tile([P, G, N], BF, name="aoT")
        wv = pers.tile([P, DT, F], BF, name="wv")
        cosTt = pers.tile([P, N], BF, name="cosTt")
        sinTt = pers.tile([P, N], BF, name="sinTt")
        cosvr = pers.tile([P, STK, DH], BF, name="cosvr")
        sinvr = pers.tile([P, STK, DH], BF, name="sinvr")
        smat = pers.tile([P, P], BF, name="smat")
        nzt = pers.tile([1, 2], BF, name="nzt")
        nzb = pers.tile([P, 2], BF, name="nzb")
        warm = pers.tile([1, 8], F32, name="warm")
        rcfp = pers.tile([32, F], F32, name="rcfp")
        nc.vector.memset(rcfp[:], 0.0)

        def load_consts():
            """emitted after the first stationary-weight loads: DMA order =
            first-proj-chain dependencies first, so the PE starts ~5us in
            instead of waiting behind all the constants."""
            nc.sync.dma_start(
                xt[:, :, 0:F],
                xt_d.ap()[:, 0:F].rearrange("(a p) n -> p a n", p=P))
            nc.sync.dma_start(smat[:], smat_d.ap())
            nc.sync.dma_start(cosTt[:], cosT_d.ap())
            nc.sync.dma_start(sinTt[:], sinT_d.ap())
            for sc in range(1, 4):
                sl = slice(sc * F, (sc + 1) * F)
                nc.sync.dma_start(
                    xt[:, :, sl],
                    xt_d.ap()[:, sl].rearrange("(a p) n -> p a n", p=P))
            nc.sync.dma_start(
                cosvr[:], cosv_d.ap().rearrange("(t p) d -> p t d", p=P))
            nc.sync.dma_start(
                sinvr[:], sinv_d.ap().rearrange("(t p) d -> p t d", p=P))
            nc.sync.dma_start(
                wv[:], wv_d.ap().rearrange("(a p) c -> p a c", p=P))
            nc.sync.dma_start(nzt[:], nz_d.ap())
            nc.gpsimd.partition_broadcast(nzb[:], nzt[0:1, :])
            # ones columns for the augmented-v denominator trick
            vones = vaug[:, :, 0:VW].rearrange("p t (h e) -> p t h e",
                                               e=DH + 1)
            nc.vector.memset(vones[:, :, :, DH:DH + 1], 1.0)
            # warm the exp activation table set early
            nc.vector.memset(warm[:], 0.0)
            nc.scalar.activation(warm[:], warm[:], EXPF, scale=SCALE)

        with ExitStack() as pa:
            pp = pa.enter_context(tc.tile_pool(name="pp", bufs=2, space="PSUM"))
            simp = pa.enter_context(
                tc.tile_pool(name="simp", bufs=1, space="PSUM"))
            avp = pa.enter_context(
                tc.tile_pool(name="avp", bufs=2, space="PSUM"))
            wkp = pa.enter_context(tc.tile_pool(name="wkp", bufs=2))
            rp = pa.enter_context(tc.tile_pool(name="rp", bufs=2))
            ep = pa.enter_context(tc.tile_pool(name="ep", bufs=2))
            np_ = pa.enter_context(tc.tile_pool(name="npool", bufs=1))
            gp = pa.enter_context(tc.tile_pool(name="gp", bufs=1))

            if exchange:
                dram = pa.enter_context(
                    tc.tile_pool(name="dram", bufs=1, space="DRAM"))
                # flat layout is slot-major: rows 0:128 = pair member 0
                gat_in = dram.tile([P, G, NQ], BF, tag="gin", name="gin")
                gat_out = dram.tile([2 * P, G, NQ], BF, tag="gout",
                                    name="gout")

            def load_wst(w_d, g, nm):
                """stationary W tile [128 d x 8 a x 128 cols] for group g"""
                w = wkp.tile([P, DT, P], BF, tag="wk", name=nm)
                nc.sync.dma_start(
                    w[:],
                    w_d.ap()[:, g * P:(g + 1) * P].rearrange(
                        "(a p) c -> p a c", p=P))
                return w

            def proj_chunk(w, sc, dstT, g):
                """one [128 dims x 512 seq] projected+rotated chunk."""
                sl = slice(sc * F, (sc + 1) * F)
                ps = pp.tile([P, F], F32, tag="pp", name="ps")
                for a in range(DT):
                    nc.tensor.matmul(
                        ps, w[:, a, :], xt[:, a, sl],
                        start=(a == 0), stop=(a == DT - 1))
                raw = rp.tile([P, F], BF, tag="raw", name="raw")
                nc.vector.tensor_copy(raw[:], ps)
                pr = pp.tile([P, F], F32, tag="pp", name="pr")
                nc.tensor.matmul(pr, smat[:], raw[:], start=True, stop=True)
                t1 = rp.tile([P, F], BF, tag="t1", name="t1")
                t2 = rp.tile([P, F], BF, tag="t2", name="t2")
                nc.vector.tensor_mul(t1[:], raw[:], cosTt[:, sl])
                nc.vector.tensor_mul(t2[:], pr, sinTt[:, sl])
                nc.vector.tensor_add(dstT[:, g, sl], t1[:], t2[:])

            def v_tile(st):
                """project+rotate v for seq tile st, all 8 local heads."""
                ps = pp.tile([P, F], F32, tag="pp", name="vps")
                for a in range(DT):
                    nc.tensor.matmul(
                        ps, xt[:, a, st * P:(st + 1) * P], wv[:, a, :],
                        start=(a == 0), stop=(a == DT - 1))
                psv = ps.rearrange("p (h d) -> p h d", d=DH)
                co = cosvr[:, st:st + 1, :].broadcast_to([P, HL, DH])
                silo = sinvr[:, st:st + 1, 0:32].broadcast_to([P, HL, 32])
                sihi = sinvr[:, st:st + 1, 32:64].broadcast_to([P, HL, 32])
                t1 = rp.tile([P, HL, DH], BF, tag="vt1", name="vt1")
                t2 = rp.tile([P, HL, DH], BF, tag="vt2", name="vt2")
                nc.vector.tensor_mul(t1[:], psv, co)
                nc.vector.tensor_mul(t2[:, :, 0:32], psv[:, :, 32:64], silo)
                nc.vector.tensor_mul(t2[:, :, 32:64], psv[:, :, 0:32], sihi)
                vview = vaug[:, st, 0:VW].rearrange("p (h e) -> p h e", e=DH + 1)
                nc.vector.tensor_add(vview[:, :, 0:DH], t1[:], t2[:])

            def sim_kt(p, qc, kt, et):
                """one kt: row-tiled sim pair + exp. A/B tile ping-pong by
                kt parity so ACT(kt) overlaps the sims of kt+1. The two
                K=64 matmuls run CONCURRENTLY in the PE array
                (tile_position (0,0)/(64,0) auto-derived)."""
                qs = slice(qc * F, (qc + 1) * F)
                ks = slice(kt * P, (kt + 1) * P)
                sp = simp.tile([P, 2, F], F32,
                               tag="simA" if kt % 2 == 0 else "simB",
                               name="sp")
                nc.tensor.matmul(
                    sp[:, 0, :], kT[0:64, p, ks], qT[0:64, p, qs],
                    start=True, stop=True)
                nc.tensor.matmul(
                    sp[:, 1, :], kT[64:128, p, ks], qT[64:128, p, qs],
                    start=True, stop=True)
                nc.scalar.activation(
                    et[:, kt, :, :], sp[:], EXPF, scale=SCALE)

            def av_mms(p, kt, et, aps):
                """one kt step of both heads' augmented-v chains."""
                for hh in range(2):
                    h = 2 * p + hh
                    nc.tensor.matmul(
                        aps[hh][0:DH + 1, :],
                        vaug[:, kt, h * (DH + 1):h * (DH + 1) + DH + 1],
                        et[:, kt, hh, :],
                        start=(kt == 0), stop=(kt == STK - 1))

            def normalize(p, qc, aps, shuffle=False):
                for hh in range(2):
                    ap_ = aps[hh]
                    # partition-shifted DVE ops only work as plain copies on
                    # HW: land the denom on partition 0, then reciprocal
                    sc = np_.tile([1, F], F32, tag="sc", name="sc")
                    nc.vector.tensor_copy(sc[:], ap_[DH:DH + 1, :])
                    rbc = np_.tile([DH, F], F32, tag="rbc", name="rbc")
                    if shuffle:
                        # blocks inside the collective's ~45us gpsimd
                        # blockage use a 2-step DVE shuffle broadcast
                        nc.vector.reciprocal_approx_fast(rcfp[0:1, :], sc[:])
                        nc.vector.stream_shuffle(
                            rbc[0:32, :], rcfp[0:32, :], [0] * 32)
                        nc.vector.stream_shuffle(
                            rbc[32:64, :], rbc[0:32, :], list(range(32)))
                    else:
                        rcf = np_.tile([1, F], F32, tag="rcf", name="rcf")
                        nc.vector.reciprocal_approx_fast(rcf[:], sc[:])
                        nc.gpsimd.partition_broadcast(rbc[:], rcf[0:1, :])
                    nc.vector.tensor_mul(
                        aoT[DH * hh:DH * hh + DH, p, qc * F:(qc + 1) * F],
                        ap_[0:DH, :], rbc[:])

            # ---- deferred work stream: v proj + proj pairs 1-3 ----------
            # pumped piecewise between attention matmuls so the PE never
            # idles while ACT catches up on exps
            from collections import deque
            stream = deque()
            done_labels = set()

            def pump(n):
                k = 0
                while stream and k < n:
                    lab, fn = stream.popleft()
                    if fn is not None:
                        fn()
                        k += 1
                    if lab:
                        done_labels.add(lab)

            def pump_until(label):
                while label not in done_labels and stream:
                    lab, fn = stream.popleft()
                    if fn is not None:
                        fn()
                    if lab:
                        done_labels.add(lab)

            def queue_v(st):
                cell = [None]

                def mk(a0):
                    def go():
                        if a0 == 0:
                            cell[0] = pp.tile([P, F], F32, tag="pp",
                                              name="vps")
                        for a in (a0, a0 + 1):
                            nc.tensor.matmul(
                                cell[0], xt[:, a, st * P:(st + 1) * P],
                                wv[:, a, :],
                                start=(a == 0), stop=(a == DT - 1))
                    return go

                for a0 in range(0, DT, 2):
                    stream.append(("", mk(a0)))

                def tail():
                    ps = cell[0]
                    psv = ps.rearrange("p (h d) -> p h d", d=DH)
                    co = cosvr[:, st:st + 1, :].broadcast_to([P, HL, DH])
                    silo = sinvr[:, st:st + 1, 0:32].broadcast_to([P, HL, 32])
                    sihi = sinvr[:, st:st + 1, 32:64].broadcast_to(
                        [P, HL, 32])
                    t1 = rp.tile([P, HL, DH], BF, tag="vt1", name="vt1")
                    t2 = rp.tile([P, HL, DH], BF, tag="vt2", name="vt2")
                    nc.vector.tensor_mul(t1[:], psv, co)
                    nc.vector.tensor_mul(t2[:, :, 0:32], psv[:, :, 32:64],
                                         silo)
                    nc.vector.tensor_mul(t2[:, :, 32:64], psv[:, :, 0:32],
                                         sihi)
                    vview = vaug[:, st, 0:VW].rearrange(
                        "p (h e) -> p h e", e=DH + 1)
                    nc.vector.tensor_add(vview[:, :, 0:DH], t1[:], t2[:])
                stream.append(("", tail))

            def queue_proj(tgt, w_d, g, dstT):
                cur = [None]

                def load():
                    w = wkp.tile([P, DT, P], BF, tag="wk", name=f"w{tgt}{g}")
                    nc.sync.dma_start(
                        w[:], w_d.ap()[:, g * P:(g + 1) * P].rearrange(
                            "(a p) c -> p a c", p=P))
                    cur[0] = w
                stream.append(("", load))
                for sc2 in range(4):
                    sl = slice(sc2 * F, (sc2 + 1) * F)
                    cell = [None]

                    def mk(a0, sl=sl, cell=cell):
                        def go():
                            if a0 == 0:
                                cell[0] = pp.tile([P, F], F32, tag="pp",
                                                  name="ps")
                            for a in (a0, a0 + 1):
                                nc.tensor.matmul(
                                    cell[0], cur[0][:, a, :], xt[:, a, sl],
                                    start=(a == 0), stop=(a == DT - 1))
                        return go

                    for a0 in range(0, DT, 2):
                        stream.append(("", mk(a0)))

                    def tail(sl=sl, cell=cell):
                        ps = cell[0]
                        raw = rp.tile([P, F], BF, tag="raw", name="raw")
                        nc.vector.tensor_copy(raw[:], ps)
                        pr = pp.tile([P, F], F32, tag="pp", name="pr")
                        nc.tensor.matmul(pr, smat[:], raw[:],
                                         start=True, stop=True)
                        t1 = rp.tile([P, F], BF, tag="t1", name="t1")
                        t2 = rp.tile([P, F], BF, tag="t2", name="t2")
                        nc.vector.tensor_mul(t1[:], raw[:], cosTt[:, sl])
                        nc.vector.tensor_mul(t2[:], pr, sinTt[:, sl])
                        nc.vector.tensor_add(dstT[:, g, sl], t1[:], t2[:])
                    stream.append(("", tail))

            for st in range(STK):
                queue_v(st)
            stream.append(("v", None))
            for p2 in range(1, 4):
                queue_proj("k", wk_d, p2, kT)
                queue_proj("q", wq_d, p2, qT)
                stream.append((f"proj{p2}", None))

            # ---- preamble: pair 0 q/k projections (direct) ---------------
            k0 = load_wst(wk_d, 0, "wk0")
            q0 = load_wst(wq_d, 0, "wq0")
            load_consts()
            for sc in range(4):
                proj_chunk(k0, sc, kT, 0)
            for sc in range(4):
                proj_chunk(q0, sc, qT, 0)

            # ---- pipelined attention: block bi's sims overlap block
            # bi-1's av chains kt-by-kt; stream pumped in the gaps --------
            # send half (qc 0,1) fully precedes the staging point
            BLOCKS = [(0, 0), (0, 1), (0, 2), (1, 0), (1, 1), (2, 0), (2, 1),
                      (3, 0), (3, 1), (0, 3), (1, 2), (1, 3), (2, 2), (2, 3),
                      (3, 2), (3, 3)]
            prev = None
            for bi, (p, qc) in enumerate(BLOCKS):
                if qc == 0 and p >= 1:
                    pump_until(f"proj{p}")
                if bi == 1:
                    pump_until("v")
                et = ep.tile([P, STK, 2, F], BF, tag="et", name=f"et{bi}")
                if prev is not None:
                    aps = (avp.tile([P, F], F32, tag="av", name="av0"),
                           avp.tile([P, F], F32, tag="av", name="av1"))
                for j in range(STK // 2):
                    for kt in (2 * j, 2 * j + 1):
                        if prev is not None:
                            av_mms(prev[0], kt, prev[2], aps)
                        sim_kt(p, qc, kt, et)
                    pump(4 if bi < 3 else 1)
                if prev is not None:
                    # normalizes inside the collective's ~45us gpsimd
                    # blockage window use the DVE shuffle path
                    normalize(prev[0], prev[1], aps,
                              shuffle=(prev[0], prev[1]) in ((0, 3), (1, 2)))
                    if exchange and prev[0] == 3 and prev[1] == 1:
                        # send half staged as soon as (3,1) is normalized
                        nc.sync.dma_start(gat_in[:], aoT[:, :, 0:NQ])
                        nc.gpsimd.collective_compute(
                            "AllGather", mybir.AluOpType.bypass,
                            replica_groups=GROUPS,
                            ins=[gat_in.opt()], outs=[gat_out.opt()])
                prev = (p, qc, et)
            # last block's av chains
            aps = (avp.tile([P, F], F32, tag="av", name="av0"),
                   avp.tile([P, F], F32, tag="av", name="av1"))
            for kt in range(STK):
                av_mms(prev[0], kt, prev[2], aps)
                pump(1)
            normalize(prev[0], prev[1], aps)
            pump_until("proj3")  # drain any stream leftovers

            # ---- readback + masked blend into the aoT send-half region --
            # recv[:, a, :] = gat_out[0]*nz[0] + gat_out[1]*nz[1];
            # nz = [j != hg] selects the partner's slot
            HQ = NQ // 2
            for a in range(G):
                for hf in range(2):
                    if not exchange:
                        # loopback - aoT send half IS the recv data
                        continue
                    sl = slice(hf * HQ, (hf + 1) * HQ)
                    g0 = gp.tile([P, HQ], BF, tag="g0", name="g0")
                    g1 = gp.tile([P, HQ], BF, tag="g1", name="g1")
                    nc.sync.dma_start(g0[:], gat_out[0:P, a, sl])
                    nc.sync.dma_start(g1[:], gat_out[P:2 * P, a, sl])
                    m0 = gp.tile([P, HQ], BF, tag="m0", name="m0")
                    m1 = gp.tile([P, HQ], BF, tag="m1", name="m1")
                    nc.vector.tensor_mul(
                        m0[:], g0[:], nzb[:, 0:1].broadcast_to([P, HQ]))
                    nc.vector.tensor_mul(
                        m1[:], g1[:], nzb[:, 1:2].broadcast_to([P, HQ]))
                    nc.vector.tensor_add(aoT[:, a, sl], m0[:], m1[:])

        # ---- output projection for the keep half ---------------------
        with ExitStack() as po:
            pp2 = po.enter_context(tc.tile_pool(name="pp2", bufs=2,
                                                space="PSUM"))
            wop = po.enter_context(tc.tile_pool(name="wop", bufs=1))
            ob = po.enter_context(tc.tile_pool(name="ob", bufs=2))
            wout = wop.tile([P, DT, D], BF, name="wout")
            for a in range(DT):
                nc.sync.dma_start(
                    wout[:, a, :], wout_d.ap()[a * P:(a + 1) * P, :])
            for qt in range(8):
                for ch in range(2):
                    ps = pp2.tile([P, F], F32, tag="pp2", name="opt")
                    for a in range(DT):
                        if a < G:
                            sta = aoT[:, a, NQ + qt * P:NQ + (qt + 1) * P]
                        else:
                            sta = aoT[:, a - G, qt * P:(qt + 1) * P]
                        nc.tensor.matmul(
                            ps, sta, wout[:, a, ch * F:(ch + 1) * F],
                            start=(a == 0), stop=(a == DT - 1))
                    o = ob.tile([P, F], F32, tag="o", name="ot")
                    nc.vector.tensor_copy(o[:], ps)
                    nc.sync.dma_start(
                        out_d.ap()[qt * P:(qt + 1) * P,
                                   ch * F:(ch + 1) * F], o[:])
    nc.compile()
    return nc


def prep_inputs(x, rotary_pos_emb, W_qkv, W_out):
    """Per-core input maps. Core c = b*2 + hg: batch b, heads hg*8..hg*8+8.
    Local seq order = [half-to-send, half-to-keep] (host-side swap)."""
    freqs = np.asarray(rotary_pos_emb, dtype=np.float32)
    cos = np.cos(freqs)
    sin = np.sin(freqs)
    # natural-layout (v) sign folding: rot-half source sign
    sin_v = sin.copy()
    sin_v[:, 0:32] = -sin_v[:, 0:32]
    # transposed-layout (q/k) cos/sin: [128 dims, n], sign folded per row
    dmod = np.arange(P) % DH
    sgn = np.where(dmod < 32, -1.0, 1.0).astype(np.float32)
    cosT_full = cos.T[dmod, :]                        # [128, N]
    sinT_full = (sin.T[dmod, :]) * sgn[:, None]       # [128, N]
    # permutation matrix: rp[d] = raw[sigma(d)], sigma swaps 32-halves
    sig = (np.arange(P) // DH) * DH + ((np.arange(P) % DH) + 32) % DH
    smat = np.zeros((P, P), np.float32)
    smat[sig, np.arange(P)] = 1.0

    x = np.asarray(x, dtype=np.float32)
    W_qkv = np.asarray(W_qkv, dtype=np.float32)
    W_out = np.asarray(W_out, dtype=np.float32)
    in_maps = []
    for c in range(8):
        b, hg = c // 2, c % 2
        # local order: send half first (global other half), keep half second
        if hg == 0:
            idx = np.r_[NQ:N, 0:NQ]
        else:
            idx = np.r_[0:N]
        cs = slice(hg * (HL * DH), (hg + 1) * (HL * DH))
        m = {
            "xt": np.ascontiguousarray(x[b].T[:, idx]).astype(bf16),
            "cosT": np.ascontiguousarray(cosT_full[:, idx]).astype(bf16),
            "sinT": np.ascontiguousarray(sinT_full[:, idx]).astype(bf16),
            "cosv": np.ascontiguousarray(cos[idx]).astype(bf16),
            "sinv": np.ascontiguousarray(sin_v[idx]).astype(bf16),
            "smat": smat.astype(bf16),
            "nz": np.array([[1.0 - (0 == hg), 1.0 - (1 == hg)]],
                           np.float32).astype(bf16),
            "wq": np.ascontiguousarray(W_qkv[:, 0 * D:1 * D][:, cs]
                                       ).astype(bf16),
            "wk": np.ascontiguousarray(W_qkv[:, 1 * D:2 * D][:, cs]
                                       ).astype(bf16),
            "wv": np.ascontiguousarray(W_qkv[:, 2 * D:3 * D][:, cs]
                                       ).astype(bf16),
            "wout": np.ascontiguousarray(np.concatenate(
                [W_out[hg * 512:(hg + 1) * 512, :],
                 W_out[(1 - hg) * 512:(2 - hg) * 512, :]], axis=0)
            ).astype(bf16),
        }
        in_maps.append(m)
    return in_maps


def kernel(x, mask, rotary_pos_emb, W_qkv, W_out):
    global _CACHED_NC
    from concourse.bass_utils import run_bass_kernel_spmd

    if _CACHED_NC is None:
        _CACHED_NC = build_nc(exchange=True)
    nc = _CACHED_NC

    in_maps = prep_inputs(x, rotary_pos_emb, W_qkv, W_out)
    res = run_bass_kernel_spmd(nc, in_maps, core_ids=list(range(8)))
    out = np.empty((B, N, D), dtype=np.float32)
    for c in range(8):
        b, hg = c // 2, c % 2
        out[b, hg * NQ:(hg + 1) * NQ, :] = res.results[c]["out"]
    return out
